# revision 5
# baseline (speedup 1.0000x reference)
"""ExtendedMoCHILoss on 8 Trainium2 NeuronCores (Bass/Tile) — v2.

Strategy (memory-bound; DMA fabric caps ~300 GB/s/core):
  - Rows are quantized to bf16 on the host and uploaded TRANSPOSED
    ([D, rows]); this halves the DMA floor vs f32 (~30us/core).  The loss
    is log-dominated (loss ~ log S), so bf16 row error (~0.4%/elem)
    perturbs the loss by only ~1e-4 relative — far inside the 2e-2 gate.
  - Column-output matmuls: for each 128-row group g and d-chunk c,
        dot col:  matmul(out=[128,1], lhsT=x_c[:, g*128:(g+1)*128], rhs=a_c)
        ssq col:  matmul(out=[128,1], lhsT=(x_c*x_c),               rhs=ones)
    accumulated over the 4 d-chunks into two [128, 72] PSUM tiles
    (64 h-groups + 8 p-groups).  Results land directly in the
    tail-friendly [128, cols] layout: no [1,512] PSUM strips, no
    single-partition evictions, no DRAM bounce (the v1 kernel spent
    ~25us of DVE time there).
  - DVE squares run in bf16 4x mode (~10us total, hidden under DMA).
  - Tail: positives stream first; their logits AllGather (4KB) is issued
    ~5us in and hides under the h-stream.  Each core then computes the
    full positive term after a tiny 32B AllGather of per-core negative
    exp-sums.  h-side 1/||row|| uses a DVE-only Newton rsqrt so ACT can
    sit on the Exp table across the stream->tail boundary (a table
    switch costs 1.28us); the Ln table is preloaded by a dummy op that
    executes during the AllGather wait.
  - The 128 synthesized negatives (MoCHi mixes) use 192 host-gathered
    rows in f32, closed-form on cosines (see _synth_block); only core 0
    counts their exp-sum via a mask input.
"""

import os
import sys

sys.path.insert(0, "/opt/trn_rl_repo")

import numpy as np
import ml_dtypes

import concourse.bass as bass
import concourse.bacc as bacc
import concourse.tile as tile
from concourse import mybir
from concourse.bass_utils import run_bass_kernel_spmd

N_CORES = 8
D = 512
N_POS = 8192
N_HARD = 65536
N_MIX = 64
HS = N_HARD // N_CORES  # 8192 h rows per core
PS = N_POS // N_CORES  # 1024 p rows per core
P = 128
HG = HS // P  # 64 h groups per core
PG = PS // P  # 8 p groups per core
NG = HG + PG  # 72 psum columns
INV_TAU = 10.0
EPS_DENOM = 1e-8
EPS_NSQ = 1e-24

F32 = mybir.dt.float32
BF16 = mybir.dt.bfloat16
I32 = mybir.dt.int32
ActF = mybir.ActivationFunctionType
Alu = mybir.AluOpType
AXX = mybir.AxisListType.X

MAGIC = 0x5F3759DF

_CACHED_NC = None


def _bcast_ap(ap, parts):
    """Partition-broadcast read of a single-partition DRAM AP."""
    return bass.AP(tensor=ap.tensor, offset=ap.offset, ap=[[0, parts], ap.ap[1]])


def _build(loops=0, serial=False):
    """loops=0: single-shot correctness/production program.
    loops=K>0: wrap the whole per-iteration body (stream + collectives +
    tail) in a tc.For_i(0, K) hardware loop for wall-clock calibration.
    serial=True: fence each iteration on the previous loss write so the
    loop measures single-shot latency instead of pipelined throughput."""
    nc = bacc.Bacc("TRN2", target_bir_lowering=False, debug=False, num_devices=N_CORES)

    hs = nc.dram_tensor("hs", [D, HS], BF16, kind="ExternalInput").ap()  # transposed
    ps = nc.dram_tensor("ps", [D, PS], BF16, kind="ExternalInput").ap()  # transposed
    anc = nc.dram_tensor("anc", [1, D], F32, kind="ExternalInput").ap()
    gmix = nc.dram_tensor("gmix", [N_MIX, D], F32, kind="ExternalInput").ap()
    gxa = nc.dram_tensor("gxa", [N_MIX, D], F32, kind="ExternalInput").ap()
    gxb = nc.dram_tensor("gxb", [N_MIX, D], F32, kind="ExternalInput").ap()
    araw = nc.dram_tensor("araw", [N_MIX, 1], F32, kind="ExternalInput").ap()
    braw = nc.dram_tensor("braw", [N_MIX, 1], F32, kind="ExternalInput").ap()
    mask = nc.dram_tensor("mask", [1, 1], F32, kind="ExternalInput").ap()
    loss = nc.dram_tensor("loss", [1, 1], F32, kind="ExternalOutput").ap()

    with tile.TileContext(nc) as tc:
        with (
            tc.tile_pool(name="stream", bufs=3) as stream,  # h half-chunk tiles
            tc.tile_pool(name="sqp", bufs=3) as sqp,  # squared half-chunks
            tc.tile_pool(name="sqscr", bufs=2) as sqscr,  # ACT scratch
            tc.tile_pool(name="prod", bufs=2) as prod_pool,  # synth muls
            tc.tile_pool(name="tsscr", bufs=2) as tsscr,  # synth reduce scratch
            tc.tile_pool(name="single", bufs=1) as single,  # persistent small
            tc.tile_pool(name="psum", bufs=1, space="PSUM") as psum,
            tc.tile_pool(name="dram", bufs=1, space="DRAM") as dram,
        ):

            def ts_rowsum(dst_col, src, scr_tag="tss"):
                """dst_col[*,1] = rowsum(src) via DVE tensor_scalar*1.0."""
                scr = tsscr.tile(list(src.shape), F32, tag=scr_tag, name="tsscr")
                nc.vector.tensor_scalar(
                    out=scr,
                    in0=src,
                    scalar1=1.0,
                    scalar2=None,
                    op0=Alu.mult,
                    op1=Alu.add,
                    accum_out=dst_col,
                )

            # ---------- persistent setup (outside any calibration loop) ----
            ab = single.tile([P, D], F32, tag="ab")  # raw anchor broadcast
            nc.scalar.dma_start(out=ab, in_=_bcast_ap(anc, P))
            mask_col = single.tile([P, 1], F32, tag="maskc")
            nc.scalar.dma_start(out=mask_col, in_=_bcast_ap(mask, P))

            # avt: partition p of column c holds anchor[0, c*128+p]
            avt = single.tile([P, 4], F32, tag="avt")
            avt_src = bass.AP(
                tensor=anc.tensor, offset=anc.offset, ap=[[1, P], [P, 4]]
            )
            nc.scalar.dma_start(out=avt, in_=avt_src)
            avt_b = single.tile([P, 4], BF16, tag="avtb")
            nc.vector.tensor_copy(out=avt_b, in_=avt)

            al_raw = single.tile([N_MIX, 1], F32, tag="alraw")
            nc.scalar.dma_start(out=al_raw, in_=araw)
            be_raw = single.tile([N_MIX, 1], F32, tag="beraw")
            nc.scalar.dma_start(out=be_raw, in_=braw)
            gtiles = {}
            for name, src in (("A", gmix), ("B", gxa), ("C", gxb)):
                gt = single.tile([N_MIX, D], F32, tag=f"g{name}", name=f"g{name}")
                nc.scalar.dma_start(out=gt, in_=src)
                gtiles[name] = gt

            # sumsq(anchor) on every partition; s_col = rsqrt(aa)/tau
            aa_scr = sqscr.tile([P, D], F32, tag="sq")
            aa = single.tile([P, 1], F32, tag="aa")
            nc.scalar.activation(out=aa_scr, in_=ab, func=ActF.Square, accum_out=aa)
            nc.vector.tensor_scalar_max(out=aa, in0=aa, scalar1=EPS_NSQ)
            na = single.tile([P, 1], F32, tag="na")
            nc.scalar.sqrt(out=na, in_=aa)
            inv_na = single.tile([P, 1], F32, tag="invna")
            nc.vector.reciprocal(out=inv_na, in_=na)
            s_col = single.tile([P, 1], F32, tag="scol")
            nc.vector.tensor_scalar_mul(out=s_col, in0=inv_na, scalar1=INV_TAU)

            ones = single.tile([P, 1], F32, tag="ones")
            nc.vector.memset(ones, 1.0)
            ones_b = single.tile([P, 1], BF16, tag="onesb")
            nc.vector.memset(ones_b, 1.0)
            negp = single.tile([1, 8], F32, tag="negp")
            nc.vector.memset(negp, 0.0)

            # DRAM collective buffers
            ag1_in = dram.tile([1, PS], F32, tag="ag1in")
            ag1_out = dram.tile([1, PS * N_CORES], F32, tag="ag1out")
            ag2_in = dram.tile([1, 8], F32, tag="ag2in")
            ag2_out = dram.tile([1, 8 * N_CORES], F32, tag="ag2out")

            def body():
                if serial:
                    # Latency fence: the sync DMA queue stalls here until the
                    # previous iteration's loss lands, so iterations do not
                    # pipeline and the loop measures single-shot latency.
                    fence = single.tile([1, 1], F32, tag="fence")
                    nc.sync.dma_start(out=fence, in_=loss)
                # ---------- positives stream (first; feeds hidden AG#1) ----
                dotps = psum.tile([P, NG], F32, tag="dotps", name="dotps")
                ssqps = psum.tile([P, NG], F32, tag="ssqps", name="ssqps")
                pch = []
                for c in range(4):
                    pc_t = stream.tile(
                        [P, PS], BF16, tag="pch", name=f"pch{c}", bufs=4
                    )
                    nc.sync.dma_start(out=pc_t, in_=ps[c * P : (c + 1) * P, :])
                    pch.append(pc_t)
                for c in range(4):
                    pq = sqp.tile([P, PS], BF16, tag="pq", name=f"pq{c}", bufs=2)
                    nc.vector.tensor_mul(out=pq, in0=pch[c], in1=pch[c])
                    for g in range(PG):
                        sl = slice(g * P, (g + 1) * P)
                        nc.tensor.matmul(
                            dotps[:, HG + g : HG + g + 1],
                            lhsT=pch[c][:, sl],
                            rhs=avt_b[:, c : c + 1],
                            start=(c == 0),
                            stop=(c == 3),
                            skip_group_check=True,
                        )
                        nc.tensor.matmul(
                            ssqps[:, HG + g : HG + g + 1],
                            lhsT=pq[:, sl],
                            rhs=ones_b[:, 0:1],
                            start=(c == 0),
                            stop=(c == 3),
                            skip_group_check=True,
                        )

                # p-tail: lp = dot * rsqrt(ssq) * s_col  -> ag1_in -> AG#1
                pvn = single.tile([P, PG], F32, tag="pvn")
                nc.vector.tensor_scalar_max(
                    out=pvn, in0=ssqps[:, HG:NG], scalar1=EPS_NSQ
                )
                nc.scalar.sqrt(out=pvn, in_=pvn)
                nc.vector.reciprocal(out=pvn, in_=pvn)
                lp = single.tile([P, PG], F32, tag="lp")
                nc.vector.tensor_mul(out=lp, in0=dotps[:, HG:NG], in1=pvn)
                nc.vector.tensor_scalar_mul(out=lp, in0=lp, scalar1=s_col)
                lp_dst = bass.AP(
                    tensor=ag1_in.tensor, offset=ag1_in.offset, ap=[[PG, P], [1, PG]]
                )
                nc.scalar.dma_start(out=lp_dst, in_=lp)
                nc.gpsimd.collective_compute(
                    "AllGather",
                    Alu.bypass,
                    replica_groups=[list(range(N_CORES))],
                    ins=[ag1_in.opt()],
                    outs=[ag1_out.opt()],
                )

                # ---------- h stream: 4 d-chunks x 2 column-halves ----------
                for c in range(4):
                    for half in range(2):
                        W = HS // 2
                        xt = stream.tile([P, W], BF16, tag="xt", name="xt")
                        nc.sync.dma_start(
                            out=xt,
                            in_=hs[c * P : (c + 1) * P, half * W : (half + 1) * W],
                        )
                        xq = sqp.tile([P, W], BF16, tag="xq", name="xq")
                        nc.vector.tensor_mul(out=xq, in0=xt, in1=xt)
                        for gi in range(W // P):
                            g = half * (W // P) + gi
                            sl = slice(gi * P, (gi + 1) * P)
                            nc.tensor.matmul(
                                dotps[:, g : g + 1],
                                lhsT=xt[:, sl],
                                rhs=avt_b[:, c : c + 1],
                                start=(c == 0),
                                stop=(c == 3),
                                skip_group_check=True,
                            )
                            nc.tensor.matmul(
                                ssqps[:, g : g + 1],
                                lhsT=xq[:, sl],
                                rhs=ones_b[:, 0:1],
                                start=(c == 0),
                                stop=(c == 3),
                                skip_group_check=True,
                            )

                # ---------- synthesized negatives (overlap h stream) -------
                # Closed form on cosines (h_n = row/||row||, av = a/||a||):
                #   hardest: u = (1-al)*h_n[m] + al*av
                #     u.av  = cA + al*(1-cA);  |u|^2 = 1 - 2 al (1-al)(1-cA)
                #   harder:  v = be*h_n[x] + (1-be)*h_n[y]
                #     v.av  = cC + be*(cB-cC); |v|^2 = 1 - 2 be (1-be)(1-cBC)
                gss = {}
                gdot = {}
                for name, gt in gtiles.items():
                    scr = sqscr.tile([N_MIX, D], F32, tag="sq64", name="sq64")
                    ss = single.tile([N_MIX, 1], F32, tag=f"ss{name}", name="ss")
                    nc.scalar.activation(
                        out=scr, in_=gt, func=ActF.Square, accum_out=ss
                    )
                    gss[name] = ss
                    pr = prod_pool.tile([N_MIX, D], F32, tag="prod64", name="pr")
                    nc.vector.tensor_mul(out=pr, in0=gt, in1=ab[0:N_MIX, :])
                    dt_ = single.tile([N_MIX, 1], F32, tag=f"dot{name}", name="dt")
                    ts_rowsum(dt_, pr, "tss64")
                    gdot[name] = dt_
                prBC = prod_pool.tile([N_MIX, D], F32, tag="prod64", name="prBC")
                nc.vector.tensor_mul(out=prBC, in0=gtiles["B"], in1=gtiles["C"])
                dBC = single.tile([N_MIX, 1], F32, tag="dotBC")
                ts_rowsum(dBC, prBC, "tss64")

                ginv = {}
                for name in ("A", "B", "C"):
                    t = single.tile([N_MIX, 1], F32, tag=f"ginv{name}", name="gi")
                    nc.vector.tensor_scalar_max(
                        out=t, in0=gss[name], scalar1=EPS_NSQ
                    )
                    nc.scalar.sqrt(out=t, in_=t)
                    nc.vector.reciprocal(out=t, in_=t)
                    ginv[name] = t
                gcos = {}
                for name in ("A", "B", "C"):
                    cc = single.tile([N_MIX, 1], F32, tag=f"gcos{name}", name="gc")
                    nc.vector.tensor_mul(out=cc, in0=gdot[name], in1=ginv[name])
                    nc.vector.tensor_mul(out=cc, in0=cc, in1=inv_na[0:N_MIX, :])
                    gcos[name] = cc
                cBC = single.tile([N_MIX, 1], F32, tag="cosBC")
                nc.vector.tensor_mul(out=cBC, in0=dBC, in1=ginv["B"])
                nc.vector.tensor_mul(out=cBC, in0=cBC, in1=ginv["C"])

                spre = single.tile([N_MIX, 2], F32, tag="spre")

                def _mix_logit_pre(out_ap, coef, cdot, cmix, tagp):
                    # out = cdot * rsqrt(1 - 2*coef*(1-coef)*(1-cmix))
                    w = single.tile([N_MIX, 1], F32, tag=f"w{tagp}", name="w")
                    nc.vector.tensor_scalar(
                        out=w, in0=coef, scalar1=-1.0, scalar2=1.0,
                        op0=Alu.mult, op1=Alu.add,
                    )
                    nc.vector.tensor_mul(out=w, in0=w, in1=coef)
                    omc = single.tile([N_MIX, 1], F32, tag=f"omc{tagp}", name="omc")
                    nc.vector.tensor_scalar(
                        out=omc, in0=cmix, scalar1=-1.0, scalar2=1.0,
                        op0=Alu.mult, op1=Alu.add,
                    )
                    nsq = single.tile([N_MIX, 1], F32, tag=f"nsq{tagp}", name="nsq")
                    nc.vector.tensor_mul(out=nsq, in0=w, in1=omc)
                    nc.vector.tensor_scalar(
                        out=nsq, in0=nsq, scalar1=-2.0, scalar2=1.0,
                        op0=Alu.mult, op1=Alu.add,
                    )
                    nc.vector.tensor_scalar_max(out=nsq, in0=nsq, scalar1=EPS_NSQ)
                    nc.scalar.sqrt(out=nsq, in_=nsq)
                    nc.vector.reciprocal(out=nsq, in_=nsq)
                    nc.vector.tensor_mul(out=out_ap, in0=cdot, in1=nsq)

                al = single.tile([N_MIX, 1], F32, tag="al")
                nc.vector.tensor_scalar(
                    out=al, in0=al_raw, scalar1=0.4, scalar2=0.1,
                    op0=Alu.mult, op1=Alu.add,
                )
                udot = single.tile([N_MIX, 1], F32, tag="udot")
                nc.vector.tensor_scalar(
                    out=udot, in0=gcos["A"], scalar1=-1.0, scalar2=1.0,
                    op0=Alu.mult, op1=Alu.add,
                )
                nc.vector.tensor_mul(out=udot, in0=udot, in1=al)
                nc.vector.tensor_add(out=udot, in0=udot, in1=gcos["A"])
                _mix_logit_pre(spre[:, 0:1], al, udot, gcos["A"], "u")

                be = single.tile([N_MIX, 1], F32, tag="be")
                nc.vector.tensor_scalar(
                    out=be, in0=be_raw, scalar1=0.4, scalar2=0.3,
                    op0=Alu.mult, op1=Alu.add,
                )
                vdot = single.tile([N_MIX, 1], F32, tag="vdot")
                nc.vector.tensor_sub(out=vdot, in0=gcos["B"], in1=gcos["C"])
                nc.vector.tensor_mul(out=vdot, in0=vdot, in1=be)
                nc.vector.tensor_add(out=vdot, in0=vdot, in1=gcos["C"])
                _mix_logit_pre(spre[:, 1:2], be, vdot, cBC, "v")

                sexp_scr = sqscr.tile([N_MIX, 2], F32, tag="sexpscr")
                ssum = single.tile([N_MIX, 1], F32, tag="ssum")
                nc.scalar.activation(
                    out=sexp_scr, in_=spre, func=ActF.Exp, scale=INV_TAU,
                    accum_out=ssum,
                )
                msynth = single.tile([N_MIX, 1], F32, tag="msynth")
                nc.vector.tensor_scalar_mul(
                    out=msynth, in0=ssum, scalar1=mask_col[0:N_MIX, :]
                )

                # ---------- AG#1 consume (overlaps h stream) ---------------
                lpa = single.tile([P, N_CORES, PG], F32, tag="lpall", name="lpa")
                lpa_src = bass.AP(
                    tensor=ag1_out.tensor,
                    offset=ag1_out.offset,
                    ap=[[PG, P], [PS, N_CORES], [1, PG]],
                )
                nc.gpsimd.dma_start(out=lpa, in_=lpa_src)
                lpa2 = lpa.rearrange("p a b -> p (a b)")
                e = single.tile([P, N_CORES * PG], F32, tag="pe", name="e")
                nc.scalar.activation(out=e, in_=lpa2, func=ActF.Exp, scale=-1.0)

                # ---------- h tail -----------------------------------------
                # DVE-only Newton rsqrt keeps ACT on the Exp table.
                qsb = single.tile([P, HG], F32, tag="qsb")
                nc.vector.tensor_copy(out=qsb, in_=ssqps[:, 0:HG])
                hq = single.tile([P, HG], F32, tag="hq")
                nc.vector.tensor_scalar_mul(out=hq, in0=qsb, scalar1=0.5)
                y = single.tile([P, HG], F32, tag="invh")
                yi = y.bitcast(I32)
                nc.vector.tensor_scalar(
                    out=yi, in0=qsb.bitcast(I32), scalar1=1, scalar2=None,
                    op0=Alu.logical_shift_right,
                )
                nc.vector.tensor_scalar(
                    out=yi, in0=yi, scalar1=-1, scalar2=MAGIC,
                    op0=Alu.mult, op1=Alu.add,
                )
                t_nw = single.tile([P, HG], F32, tag="tnw")
                for _ in range(2):  # two Newton steps: rel err -> ~5e-6
                    nc.vector.tensor_mul(out=t_nw, in0=y, in1=y)
                    nc.vector.tensor_mul(out=t_nw, in0=t_nw, in1=hq)
                    nc.vector.tensor_scalar(
                        out=t_nw, in0=t_nw, scalar1=-1.0, scalar2=1.5,
                        op0=Alu.mult, op1=Alu.add,
                    )
                    nc.vector.tensor_mul(out=y, in0=y, in1=t_nw)

                pre_h = single.tile([P, HG], F32, tag="preh")
                nc.vector.tensor_mul(out=pre_h, in0=dotps[:, 0:HG], in1=y)
                hexp_scr = sqscr.tile([P, HG], F32, tag="hexps")
                hsum = single.tile([P, 1], F32, tag="hsum")
                nc.scalar.activation(
                    out=hexp_scr, in_=pre_h, func=ActF.Exp, scale=s_col,
                    accum_out=hsum,
                )
                nc.vector.tensor_add(
                    out=hsum[0:N_MIX, :], in0=hsum[0:N_MIX, :], in1=msynth
                )
                negp_ps = psum.tile([1, 1], F32, tag="negps", name="negp_ps")
                nc.tensor.matmul(
                    negp_ps, lhsT=hsum, rhs=ones, start=True, stop=True
                )
                nc.vector.tensor_copy(out=negp[0:1, 0:1], in_=negp_ps)
                nc.gpsimd.dma_start(out=ag2_in, in_=negp)
                nc.gpsimd.collective_compute(
                    "AllGather",
                    Alu.bypass,
                    replica_groups=[list(range(N_CORES))],
                    ins=[ag2_in.opt()],
                    outs=[ag2_out.opt()],
                )

                # Preload the Ln table while AG#2 is in flight.
                lnscr = single.tile([1, 1], F32, tag="lnscr")
                nc.scalar.activation(out=lnscr, in_=ones[0:1, :], func=ActF.Ln)

                # ---------- finish -----------------------------------------
                negs = single.tile([P, N_CORES], F32, tag="negs")
                negs_src = bass.AP(
                    tensor=ag2_out.tensor,
                    offset=ag2_out.offset,
                    ap=[[0, P], [8, N_CORES]],
                )
                nc.gpsimd.dma_start(out=negs, in_=negs_src)
                s_eps = single.tile([P, 1], F32, tag="seps")
                nc.vector.reduce_sum(out=s_eps, in_=negs, axis=AXX)
                nc.vector.tensor_scalar_add(out=s_eps, in0=s_eps, scalar1=EPS_DENOM)

                f = single.tile([P, N_CORES * PG], F32, tag="pf", name="f")
                nc.vector.tensor_scalar_mul(out=f, in0=e, scalar1=s_eps)
                t_ln = single.tile([P, N_CORES * PG], F32, tag="pt", name="t_ln")
                pp = single.tile([P, 1], F32, tag="pp")
                nc.scalar.activation(
                    out=t_ln, in_=f, func=ActF.Ln, bias=1.0, scale=1.0,
                    accum_out=pp,
                )
                posp_ps = psum.tile([1, 1], F32, tag="posps", name="posp_ps")
                nc.tensor.matmul(posp_ps, lhsT=pp, rhs=ones, start=True, stop=True)
                lsum = single.tile([1, 1], F32, tag="lsum")
                nc.vector.tensor_scalar_mul(
                    out=lsum, in0=posp_ps, scalar1=1.0 / N_POS
                )
                nc.gpsimd.dma_start(out=loss, in_=lsum)

            if loops:
                with tc.For_i(0, loops) as _i:
                    body()
            else:
                body()

    nc.compile()
    return nc


def _get_nc():
    global _CACHED_NC
    if _CACHED_NC is None:
        _CACHED_NC = _build()
    return _CACHED_NC


LAST_RESULTS = None  # BassKernelResults of the most recent run (for profiling)


def _in_maps(anchor, h, p, gm, ga, gb, ar, br):
    h16 = h.astype(ml_dtypes.bfloat16)
    p16 = p.astype(ml_dtypes.bfloat16)
    maps = []
    for c in range(N_CORES):
        maps.append(
            {
                "hs": np.ascontiguousarray(h16[c * HS : (c + 1) * HS].T),
                "ps": np.ascontiguousarray(p16[c * PS : (c + 1) * PS].T),
                "anc": anchor,
                "gmix": gm,
                "gxa": ga,
                "gxb": gb,
                "araw": ar,
                "braw": br,
                "mask": np.asarray([[1.0 if c == 0 else 0.0]], dtype=np.float32),
            }
        )
    return maps


def kernel(
    anchor, positives, hard_negatives, mix_idx, idx_a, idx_b, alpha_raw, beta_raw
):
    nc = _get_nc()
    anchor = np.ascontiguousarray(anchor, dtype=np.float32)
    h = np.ascontiguousarray(hard_negatives, dtype=np.float32)
    p = np.ascontiguousarray(positives, dtype=np.float32)
    gm = np.ascontiguousarray(h[np.asarray(mix_idx)])
    ga = np.ascontiguousarray(h[np.asarray(idx_a)])
    gb = np.ascontiguousarray(h[np.asarray(idx_b)])
    ar = np.ascontiguousarray(alpha_raw, dtype=np.float32)
    br = np.ascontiguousarray(beta_raw, dtype=np.float32)
    maps = _in_maps(anchor, h, p, gm, ga, gb, ar, br)

    if os.environ.get("KERNEL_SIM", "0") == "1":
        from concourse import bass_interp

        sim = bass_interp.MultiCoreSim(nc, N_CORES)
        for c in range(N_CORES):
            for k, v in maps[c].items():
                sim.cores[c].tensor(k)[:] = v
        sim.simulate(check_with_hw=False)
        return np.asarray(
            sim.cores[0].tensor("loss")[0, 0], dtype=np.float32
        ).reshape(())

    trace = os.environ.get("BASS_KERNEL_TRACE", "0") == "1"
    res = run_bass_kernel_spmd(nc, maps, list(range(N_CORES)), trace=trace)
    global LAST_RESULTS
    LAST_RESULTS = res
    return np.asarray(res.results[0]["loss"][0, 0], dtype=np.float32).reshape(())
